# revision 6
# baseline (speedup 1.0000x reference)
"""Trainium2 Bass kernel for nn_DensityLoss (retrieval kNN hinge loss).

Computes mean(relu(topk_smallest_dist(x_pred, x_target, k) - 1.0)).

Strategy (8 NeuronCores, SPMD):
  - Shard x_pred rows across the 8 cores (1024 rows each).
  - Host pre-transposes both point sets to [dim, n] layout (factor 2 of the
    cross term folded into a), and precomputes -||b||^2 replicated across
    the 128 partitions.
  - Each core computes 2*a.b via TensorE (bf16 operands, fp32 PSUM accum),
    ScalarE casts PSUM to fp16 into an SBUF slab, DVE adds -||b||^2
    (fp16 2x mode, in place) giving m = 2 a.b - b2 = ||a||^2 - d^2, then an
    elementwise-max fold tree on DVE reduces 16384 targets -> 1024 strided
    chunk-maxima per row.
  - Chunk maxima [rows, 1024] DMA back to host. Host picks the top-8 chunks
    per row (guaranteed to contain the true top-5 targets: a top-5 target's
    chunk-max ranks <= 5 exactly; fp16 rounding noise is absorbed by the
    3-slot margin), rescores the 8*16 = 128 candidate targets exactly in
    float64, takes top-k, applies the hinge, and averages.
"""

import numpy as np

N_CORES = 8
N_PRED = 8192
N_TGT = 16384
DIM = 128
ROWS_PER_CORE = N_PRED // N_CORES  # 1024
ROWTILES = ROWS_PER_CORE // 128    # 8
BANK = 512                         # fp32 PSUM bank, matmul max N
GROUP = 4                          # banks per PSUM tile
N_CHUNK = N_TGT // BANK            # 32 matmul chunks per rowtile
FOLD_TO = 1024                     # chunk-max vector length after fold tree
FOLD_S = N_TGT // FOLD_TO          # 16 targets per fold chunk
TOP_CHUNKS = 8
HINGE = 1.0

_CACHE = {}


def _build_nc():
    import concourse.bacc as bacc
    import concourse.bass as bass
    import concourse.mybir as mybir
    import concourse.tile as tile

    dt = mybir.dt
    nc = bacc.Bacc(
        "TRN2",
        target_bir_lowering=False,
        debug=False,
        num_devices=N_CORES,
    )
    a_t = nc.dram_tensor("a_t", [DIM, ROWS_PER_CORE], dt.bfloat16, kind="ExternalInput")
    b_t = nc.dram_tensor("b_t", [DIM, N_TGT], dt.bfloat16, kind="ExternalInput")
    nb2 = nc.dram_tensor("nb2", [128, N_TGT], dt.float16, kind="ExternalInput")
    cmx = nc.dram_tensor(
        "cmx", [ROWTILES, 128, FOLD_TO], dt.float16, kind="ExternalOutput"
    )

    n_groups = N_CHUNK // GROUP  # 8 groups of 2048 targets per rowtile
    # Groups whose -b2 fold runs on the PE (K=1 matmul accumulate) instead of
    # a DVE tensor_add: balances PE vs DVE busy time.
    pe_b2_groups = {0, 2, 4, 6}

    with tile.TileContext(nc) as tc:
        with (
            tc.tile_pool(name="const", bufs=1) as cpool,
            tc.tile_pool(name="psum", bufs=2, space="PSUM") as ppool,
            tc.tile_pool(name="slab", bufs=2) as spool,
            tc.tile_pool(name="fold", bufs=1) as fpool,
        ):
            bt_sb = cpool.tile([DIM, N_TGT], dt.bfloat16)
            at_sb = cpool.tile([DIM, ROWS_PER_CORE], dt.bfloat16)
            nb2_sb = cpool.tile([128, N_TGT], dt.float16)
            ones_sb = cpool.tile([1, 128], dt.float16)

            # Split the big input DMAs so compute can start on early slices.
            n_dma = 8
            for s in range(n_dma):
                sl = bass.ts(s, N_TGT // n_dma)
                nc.sync.dma_start(out=bt_sb[:, sl], in_=b_t[:, sl])
                nc.sync.dma_start(out=nb2_sb[:, sl], in_=nb2[:, sl])
            nc.sync.dma_start(out=at_sb[:], in_=a_t[:])
            nc.gpsimd.memset(ones_sb[:], 1.0)

            Gg = BANK * GROUP  # 2048
            for rt in range(ROWTILES):
                lhsT = at_sb[:, bass.ts(rt, 128)]
                slab = spool.tile([128, N_TGT], dt.float16)
                for g in range(n_groups):
                    on_pe = g in pe_b2_groups
                    ps = ppool.tile([128, Gg], dt.float32)
                    for j in range(GROUP):
                        c = g * GROUP + j
                        nc.tensor.matmul(
                            ps[:, bass.ts(j, BANK)],
                            lhsT,
                            bt_sb[:, bass.ts(c, BANK)],
                            start=True,
                            stop=not on_pe,
                        )
                    if on_pe:
                        for j in range(GROUP):
                            c = g * GROUP + j
                            nc.tensor.matmul(
                                ps[:, bass.ts(j, BANK)],
                                ones_sb[:],
                                nb2_sb[0:1, bass.ts(c, BANK)],
                                start=False,
                                stop=True,
                            )
                    gsl = bass.ts(g, Gg)
                    nc.scalar.copy(slab[:, gsl], ps[:])
                    if not on_pe:
                        # m = 2 a.b - b2, in place (fp16, DVE 2x mode)
                        nc.vector.tensor_add(
                            slab[:, gsl], slab[:, gsl], nb2_sb[:, gsl]
                        )
                # fold1 in pair pieces (g, g+4) so it starts before the
                # whole slab is ready; deeper folds halve contiguously.
                f1 = fpool.tile([128, N_TGT // 2], dt.float16, tag="f1")
                for g in range(4):
                    nc.vector.tensor_max(
                        f1[:, bass.ts(g, Gg)],
                        slab[:, bass.ts(g, Gg)],
                        slab[:, bass.ts(g + 4, Gg)],
                    )
                f = f1
                w = N_TGT // 2
                while w > FOLD_TO:
                    w //= 2
                    nf = fpool.tile([128, w], dt.float16, tag=f"f{w}")
                    nc.vector.tensor_max(nf[:], f[:, 0:w], f[:, w : 2 * w])
                    f = nf
                nc.sync.dma_start(out=cmx[rt], in_=f[:])

    nc.compile()
    return nc


def _get_nc():
    if "nc" not in _CACHE:
        _CACHE["nc"] = _build_nc()
    return _CACHE["nc"]


def _host_finish(x_pred, x_target, chunk_max, k):
    """chunk_max: [N_PRED, FOLD_TO] float32 of per-chunk maxima of
    m = 2 a.b - b2. Chunk j holds targets {j + FOLD_TO*i}."""
    n = x_pred.shape[0]
    ch = np.argpartition(-chunk_max, TOP_CHUNKS, axis=1)[:, :TOP_CHUNKS]
    tid = (
        ch[:, :, None] + FOLD_TO * np.arange(FOLD_S)[None, None, :]
    ).reshape(n, TOP_CHUNKS * FOLD_S)

    a64 = x_pred.astype(np.float64)
    b64 = x_target.astype(np.float64)
    a2 = np.einsum("ij,ij->i", a64, a64)
    b2 = np.einsum("ij,ij->i", b64, b64)

    vals = np.empty((n, k))
    B = 1024
    for s in range(0, n, B):
        t = tid[s : s + B]
        bg = b64[t]  # [B, C, DIM]
        dots = np.einsum("rd,rcd->rc", a64[s : s + B], bg, optimize=True)
        d2 = a2[s : s + B, None] + b2[t] - 2.0 * dots
        vals[s : s + B] = np.partition(d2, k - 1, axis=1)[:, :k]
    d = np.sqrt(np.maximum(vals, 0.0))
    return np.float32(np.maximum(d - HINGE, 0.0).mean(dtype=np.float64))


def _host_exact(x_pred, x_target, k):
    """Exact fallback (never expected in practice)."""
    a = x_pred.astype(np.float32)
    b = x_target.astype(np.float32)
    a2 = np.sum(a * a, axis=1)[:, None]
    b2 = np.sum(b * b, axis=1)[None, :]
    out = np.empty((a.shape[0], k), np.float64)
    B = 1024
    for s in range(0, a.shape[0], B):
        d2 = a2[s : s + B] + b2 - 2.0 * (a[s : s + B] @ b.T)
        out[s : s + B] = np.partition(d2, k - 1, axis=1)[:, :k].astype(np.float64)
    d = np.sqrt(np.maximum(out, 0.0))
    return np.float32(np.maximum(d - HINGE, 0.0).mean(dtype=np.float64))


def kernel(x_pred, x_target, top_k=5, _want_results=False):
    import ml_dtypes
    from concourse.bass_utils import run_bass_kernel_spmd

    x_pred = np.asarray(x_pred, dtype=np.float32)
    x_target = np.asarray(x_target, dtype=np.float32)
    k = int(top_k)
    if (
        k > TOP_CHUNKS
        or x_pred.shape != (N_PRED, DIM)
        or x_target.shape != (N_TGT, DIM)
    ):
        return _host_exact(x_pred, x_target, k)

    nc = _get_nc()

    # Factor 2 of the cross term 2*a.b is folded into a (exact in bf16).
    a_t_full = np.ascontiguousarray(2.0 * x_pred.T).astype(ml_dtypes.bfloat16)
    b_t = np.ascontiguousarray(x_target.T).astype(ml_dtypes.bfloat16)
    b2 = np.einsum("ij,ij->i", x_target, x_target, dtype=np.float64)
    nb2 = np.broadcast_to((-b2).astype(np.float16)[None, :], (128, N_TGT))
    nb2 = np.ascontiguousarray(nb2)

    in_maps = []
    for c in range(N_CORES):
        in_maps.append(
            {
                "a_t": np.ascontiguousarray(
                    a_t_full[:, c * ROWS_PER_CORE : (c + 1) * ROWS_PER_CORE]
                ),
                "b_t": b_t,
                "nb2": nb2,
            }
        )

    res = run_bass_kernel_spmd(nc, in_maps, list(range(N_CORES)))
    chunk_max = np.concatenate(
        [
            res.results[c]["cmx"].reshape(ROWS_PER_CORE, FOLD_TO)
            for c in range(N_CORES)
        ],
        axis=0,
    ).astype(np.float32)
    out = _host_finish(x_pred, x_target, chunk_max, k)
    if _want_results:
        return out, res
    return out


# revision 8
# speedup vs baseline: 1.1834x; 1.1834x over previous
"""Trainium2 Bass kernel for nn_DensityLoss (retrieval kNN hinge loss).

Computes mean(relu(topk_smallest_dist(x_pred, x_target, k) - 1.0)).

Strategy (8 NeuronCores, SPMD):
  - Shard x_pred rows across the 8 cores (1024 rows each).
  - Host pre-transposes both point sets to [dim, n] layout (factor 2 of the
    cross term folded into a), and precomputes -||b||^2 replicated across
    the 128 partitions.
  - Each core computes 2*a.b via TensorE (bf16 operands, fp32 PSUM accum),
    ScalarE casts PSUM to fp16 into an SBUF slab, DVE adds -||b||^2
    (fp16 2x mode, in place) giving m = 2 a.b - b2 = ||a||^2 - d^2, then an
    elementwise-max fold tree on DVE reduces 16384 targets -> 1024 strided
    chunk-maxima per row.
  - Chunk maxima [rows, 1024] DMA back to host. Host picks the top-8 chunks
    per row (guaranteed to contain the true top-5 targets: a top-5 target's
    chunk-max ranks <= 5 exactly; fp16 rounding noise is absorbed by the
    3-slot margin), rescores the 8*16 = 128 candidate targets exactly in
    float64, takes top-k, applies the hinge, and averages.
"""

import numpy as np

N_CORES = 8
N_PRED = 8192
N_TGT = 16384
DIM = 128
ROWS_PER_CORE = N_PRED // N_CORES  # 1024
ROWTILES = ROWS_PER_CORE // 128    # 8
BANK = 512                         # fp32 PSUM bank, matmul max N
GROUP = 4                          # banks per PSUM tile
N_CHUNK = N_TGT // BANK            # 32 matmul chunks per rowtile
FOLD_TO = 1024                     # chunk-max vector length after fold tree
FOLD_S = N_TGT // FOLD_TO          # 16 targets per fold chunk
TOP_CHUNKS = 8
HINGE = 1.0

_CACHE = {}


def _build_nc():
    import concourse.bacc as bacc
    import concourse.bass as bass
    import concourse.mybir as mybir
    import concourse.tile as tile

    dt = mybir.dt
    nc = bacc.Bacc(
        "TRN2",
        target_bir_lowering=False,
        debug=False,
        num_devices=N_CORES,
    )
    a_t = nc.dram_tensor("a_t", [DIM, ROWS_PER_CORE], dt.bfloat16, kind="ExternalInput")
    b_t = nc.dram_tensor("b_t", [DIM, N_TGT], dt.bfloat16, kind="ExternalInput")
    nb2 = nc.dram_tensor("nb2", [128, N_TGT], dt.float16, kind="ExternalInput")
    cmx = nc.dram_tensor(
        "cmx", [ROWTILES, 128, FOLD_TO], dt.float16, kind="ExternalOutput"
    )

    n_groups = N_CHUNK // GROUP  # 8 groups of 2048 targets per rowtile
    # Groups whose -b2 fold runs on the PE (K=1 matmul accumulate) instead of
    # a DVE tensor_add: balances PE vs DVE busy time. Chosen in adjacent
    # pairs so same-weight matmuls batch together (fewer LDWEIGHTS stalls).
    pe_b2_groups = {0, 1, 4}

    with tile.TileContext(nc) as tc:
        with (
            tc.tile_pool(name="const", bufs=1) as cpool,
            tc.tile_pool(name="psum", bufs=2, space="PSUM") as ppool,
            tc.tile_pool(name="slab", bufs=2) as spool,
            tc.tile_pool(name="fold", bufs=1) as fpool,
        ):
            bt_sb = cpool.tile([DIM, N_TGT], dt.bfloat16)
            at_sb = cpool.tile([DIM, ROWS_PER_CORE], dt.bfloat16)
            nb2_sb = cpool.tile([128, N_TGT], dt.float16)
            ones_sb = cpool.tile([1, 128], dt.float16)

            # Split the big input DMAs so compute can start on early slices.
            n_dma = 8
            for s in range(n_dma):
                sl = bass.ts(s, N_TGT // n_dma)
                nc.sync.dma_start(out=bt_sb[:, sl], in_=b_t[:, sl])
                nc.sync.dma_start(out=nb2_sb[:, sl], in_=nb2[:, sl])
            nc.sync.dma_start(out=at_sb[:], in_=a_t[:])
            nc.gpsimd.memset(ones_sb[:], 1.0)

            Gg = BANK * GROUP  # 2048
            for rt in range(ROWTILES):
                lhsT = at_sb[:, bass.ts(rt, 128)]
                slab = spool.tile([128, N_TGT], dt.float16)
                # Process groups in pairs: emit both groups' main matmuls
                # (stationary = lhsT) back to back, then any -b2 K=1 matmuls
                # (stationary = ones), so the PE rarely swaps weights.
                for gp in range(n_groups // 2):
                    gpair = (2 * gp, 2 * gp + 1)
                    tiles = {}
                    for g in gpair:
                        on_pe = g in pe_b2_groups
                        ps = ppool.tile([128, Gg], dt.float32)
                        tiles[g] = ps
                        for j in range(GROUP):
                            c = g * GROUP + j
                            nc.tensor.matmul(
                                ps[:, bass.ts(j, BANK)],
                                lhsT,
                                bt_sb[:, bass.ts(c, BANK)],
                                start=True,
                                stop=not on_pe,
                            )
                    for g in gpair:
                        if g not in pe_b2_groups:
                            continue
                        ps = tiles[g]
                        for j in range(GROUP):
                            c = g * GROUP + j
                            nc.tensor.matmul(
                                ps[:, bass.ts(j, BANK)],
                                ones_sb[:],
                                nb2_sb[0:1, bass.ts(c, BANK)],
                                start=False,
                                stop=True,
                            )
                    for g in gpair:
                        gsl = bass.ts(g, Gg)
                        nc.scalar.copy(slab[:, gsl], tiles[g][:])
                        if g not in pe_b2_groups:
                            # m = 2 a.b - b2, in place (fp16, DVE 2x mode)
                            nc.vector.tensor_add(
                                slab[:, gsl], slab[:, gsl], nb2_sb[:, gsl]
                            )
                # fold1 in pair pieces (g, g+4) so it starts before the
                # whole slab is ready; deeper folds halve contiguously.
                f1 = fpool.tile([128, N_TGT // 2], dt.float16, tag="f1")
                for g in range(4):
                    nc.vector.tensor_max(
                        f1[:, bass.ts(g, Gg)],
                        slab[:, bass.ts(g, Gg)],
                        slab[:, bass.ts(g + 4, Gg)],
                    )
                f = f1
                w = N_TGT // 2
                while w > FOLD_TO:
                    w //= 2
                    nf = fpool.tile([128, w], dt.float16, tag=f"f{w}")
                    nc.vector.tensor_max(nf[:], f[:, 0:w], f[:, w : 2 * w])
                    f = nf
                nc.sync.dma_start(out=cmx[rt], in_=f[:])

    nc.compile()
    return nc


def _get_nc():
    if "nc" not in _CACHE:
        _CACHE["nc"] = _build_nc()
    return _CACHE["nc"]


def _host_finish(x_pred, x_target, chunk_max, k):
    """chunk_max: [N_PRED, FOLD_TO] float32 of per-chunk maxima of
    m = 2 a.b - b2. Chunk j holds targets {j + FOLD_TO*i}."""
    n = x_pred.shape[0]
    ch = np.argpartition(-chunk_max, TOP_CHUNKS, axis=1)[:, :TOP_CHUNKS]
    tid = (
        ch[:, :, None] + FOLD_TO * np.arange(FOLD_S)[None, None, :]
    ).reshape(n, TOP_CHUNKS * FOLD_S)

    a64 = x_pred.astype(np.float64)
    b64 = x_target.astype(np.float64)
    a2 = np.einsum("ij,ij->i", a64, a64)
    b2 = np.einsum("ij,ij->i", b64, b64)

    vals = np.empty((n, k))
    B = 1024
    for s in range(0, n, B):
        t = tid[s : s + B]
        bg = b64[t]  # [B, C, DIM]
        dots = np.einsum("rd,rcd->rc", a64[s : s + B], bg, optimize=True)
        d2 = a2[s : s + B, None] + b2[t] - 2.0 * dots
        vals[s : s + B] = np.partition(d2, k - 1, axis=1)[:, :k]
    d = np.sqrt(np.maximum(vals, 0.0))
    return np.float32(np.maximum(d - HINGE, 0.0).mean(dtype=np.float64))


def _host_exact(x_pred, x_target, k):
    """Exact fallback (never expected in practice)."""
    a = x_pred.astype(np.float32)
    b = x_target.astype(np.float32)
    a2 = np.sum(a * a, axis=1)[:, None]
    b2 = np.sum(b * b, axis=1)[None, :]
    out = np.empty((a.shape[0], k), np.float64)
    B = 1024
    for s in range(0, a.shape[0], B):
        d2 = a2[s : s + B] + b2 - 2.0 * (a[s : s + B] @ b.T)
        out[s : s + B] = np.partition(d2, k - 1, axis=1)[:, :k].astype(np.float64)
    d = np.sqrt(np.maximum(out, 0.0))
    return np.float32(np.maximum(d - HINGE, 0.0).mean(dtype=np.float64))


def kernel(x_pred, x_target, top_k=5, _want_results=False):
    import ml_dtypes
    from concourse.bass_utils import run_bass_kernel_spmd

    x_pred = np.asarray(x_pred, dtype=np.float32)
    x_target = np.asarray(x_target, dtype=np.float32)
    k = int(top_k)
    if (
        k > TOP_CHUNKS
        or x_pred.shape != (N_PRED, DIM)
        or x_target.shape != (N_TGT, DIM)
    ):
        return _host_exact(x_pred, x_target, k)

    nc = _get_nc()

    # Factor 2 of the cross term 2*a.b is folded into a (exact in bf16).
    a_t_full = np.ascontiguousarray(2.0 * x_pred.T).astype(ml_dtypes.bfloat16)
    b_t = np.ascontiguousarray(x_target.T).astype(ml_dtypes.bfloat16)
    b2 = np.einsum("ij,ij->i", x_target, x_target, dtype=np.float64)
    nb2 = np.broadcast_to((-b2).astype(np.float16)[None, :], (128, N_TGT))
    nb2 = np.ascontiguousarray(nb2)

    in_maps = []
    for c in range(N_CORES):
        in_maps.append(
            {
                "a_t": np.ascontiguousarray(
                    a_t_full[:, c * ROWS_PER_CORE : (c + 1) * ROWS_PER_CORE]
                ),
                "b_t": b_t,
                "nb2": nb2,
            }
        )

    res = run_bass_kernel_spmd(nc, in_maps, list(range(N_CORES)))
    chunk_max = np.concatenate(
        [
            res.results[c]["cmx"].reshape(ROWS_PER_CORE, FOLD_TO)
            for c in range(N_CORES)
        ],
        axis=0,
    ).astype(np.float32)
    out = _host_finish(x_pred, x_target, chunk_max, k)
    if _want_results:
        return out, res
    return out


# revision 11
# speedup vs baseline: 1.4165x; 1.1970x over previous
"""Trainium2 Bass kernel for nn_DensityLoss (retrieval kNN hinge loss).

Computes mean(relu(topk_smallest_dist(x_pred, x_target, k) - 1.0)).

Strategy (8 NeuronCores, SPMD, x_pred rows sharded):
  - Host sorts targets by ||b||^2 and lays them out so each of 1024
    "fold chunks" (strided positions {j + 1024k}) holds 16 targets of
    nearly equal ||b||^2. Host pre-transposes to [dim, n] (factor 2 of the
    cross term folded into a) and precomputes per-chunk -min ||b||^2.
  - Device per core: TensorE computes 2*a.b (bf16 ops, fp32 PSUM);
    ScalarE casts PSUM into an fp16 SBUF slab; DVE runs an elementwise-max
    fold tree 16384 -> 1024 (fp16 2x mode; one fold pair reads the two PSUM
    tiles directly, skipping their ScalarE copies), then adds the per-chunk
    -b2min to the [128, 1024] tail. Result C(j) >= max_i(2 a.b_i - b2_i)
    for chunk j, and exceeds it by at most the within-chunk b2 spread
    (~0.1 typical).
  - C [rows, 1024] DMAs back to host. Host takes the top-12 chunks per row
    (a chunk holding a true top-5 target ranks <= 5 + spread-noise; 12
    leaves a wide margin), rescores the 12*16 = 192 candidates exactly in
    float64, takes top-k, hinges, averages.
"""

import numpy as np

N_CORES = 8
N_PRED = 8192
N_TGT = 16384
DIM = 128
ROWS_PER_CORE = N_PRED // N_CORES  # 1024
ROWTILES = ROWS_PER_CORE // 128    # 8
BANK = 512                         # fp32 PSUM bank, matmul max N
GROUP = 4                          # banks per PSUM tile
N_GROUPS = N_TGT // (BANK * GROUP)  # 8 groups of 2048 targets per rowtile
FOLD_TO = 1024                     # chunk vector length after fold tree
FOLD_S = N_TGT // FOLD_TO          # 16 targets per fold chunk
TOP_CHUNKS = 12
HINGE = 1.0

_CACHE = {}


def _build_nc():
    import concourse.bacc as bacc
    import concourse.bass as bass
    import concourse.mybir as mybir
    import concourse.tile as tile

    dt = mybir.dt
    nc = bacc.Bacc(
        "TRN2",
        target_bir_lowering=False,
        debug=False,
        num_devices=N_CORES,
    )
    a_t = nc.dram_tensor("a_t", [DIM, ROWS_PER_CORE], dt.bfloat16, kind="ExternalInput")
    b_t = nc.dram_tensor("b_t", [DIM, N_TGT], dt.bfloat16, kind="ExternalInput")
    nb2c = nc.dram_tensor("nb2c", [128, FOLD_TO], dt.float16, kind="ExternalInput")
    cmx = nc.dram_tensor(
        "cmx", [ROWTILES, 128, FOLD_TO], dt.float16, kind="ExternalOutput"
    )

    Gg = BANK * GROUP  # 2048 targets per PSUM tile
    # slab holds fp16 2a.b for all groups except 4: fold pair (0,4) reads
    # group 4 straight from its PSUM tile (PSUM+SBUF is the only operand
    # mix the BIR verifier accepts), skipping one ScalarE copy per rowtile.
    soff = {0: 0, 1: 2048, 2: 4096, 3: 6144, 5: 8192, 6: 10240, 7: 12288}

    with tile.TileContext(nc) as tc:
        with (
            tc.tile_pool(name="const", bufs=1) as cpool,
            tc.tile_pool(name="psum", bufs=2, space="PSUM") as ppool,
            tc.tile_pool(name="slab", bufs=2) as spool,
            tc.tile_pool(name="fold", bufs=1) as fpool,
        ):
            bt_sb = cpool.tile([DIM, N_TGT], dt.bfloat16)
            at_sb = cpool.tile([DIM, ROWS_PER_CORE], dt.bfloat16)
            nb2c_sb = cpool.tile([128, FOLD_TO], dt.float16)

            # Split the big input DMA so compute can start on early slices.
            n_dma = 8
            for s in range(n_dma):
                sl = bass.ts(s, N_TGT // n_dma)
                nc.sync.dma_start(out=bt_sb[:, sl], in_=b_t[:, sl])
            nc.sync.dma_start(out=at_sb[:], in_=a_t[:])
            nc.sync.dma_start(out=nb2c_sb[:], in_=nb2c[:])

            for rt in range(ROWTILES):
                lhsT = at_sb[:, bass.ts(rt, 128)]
                slab = spool.tile([128, 7 * Gg], dt.float16)
                f1 = fpool.tile([128, N_TGT // 2], dt.float16, tag="f1")
                tiles = {}

                def mains(g, tiles=tiles, lhsT=lhsT):
                    ps = ppool.tile([128, Gg], dt.float32)
                    tiles[g] = ps
                    for j in range(GROUP):
                        c = g * GROUP + j
                        nc.tensor.matmul(
                            ps[:, bass.ts(j, BANK)],
                            lhsT,
                            bt_sb[:, bass.ts(c, BANK)],
                            start=True,
                            stop=True,
                        )

                def evac(g, slab=slab, tiles=tiles):
                    nc.scalar.copy(
                        slab[:, soff[g] : soff[g] + Gg], tiles.pop(g)[:]
                    )

                for g in (0, 1, 2, 3):
                    mains(g)
                    evac(g)
                mains(4)
                # fold pair (0,4): group 4 read straight from PSUM
                nc.vector.tensor_max(
                    f1[:, 0:Gg], slab[:, soff[0] : soff[0] + Gg], tiles.pop(4)[:]
                )
                for g in (5, 6, 7):
                    mains(g)
                    evac(g)
                    p = g - 4
                    nc.vector.tensor_max(
                        f1[:, bass.ts(p, Gg)],
                        slab[:, soff[p] : soff[p] + Gg],
                        slab[:, soff[g] : soff[g] + Gg],
                    )
                f = f1
                w = N_TGT // 2
                while w > FOLD_TO:
                    w //= 2
                    nf = fpool.tile([128, w], dt.float16, tag=f"f{w}")
                    nc.vector.tensor_max(nf[:], f[:, 0:w], f[:, w : 2 * w])
                    f = nf
                # C = chunk-max of 2 a.b, minus per-chunk min b2
                nc.vector.tensor_add(f[:], f[:], nb2c_sb[:])
                nc.sync.dma_start(out=cmx[rt], in_=f[:])

    nc.compile()
    return nc


def _get_nc():
    if "nc" not in _CACHE:
        _CACHE["nc"] = _build_nc()
    return _CACHE["nc"]


def _prep(x_pred, x_target):
    """Host-side layout: sort targets by b2, stride into fold chunks."""
    import ml_dtypes

    b2 = np.einsum("ij,ij->i", x_target.astype(np.float64), x_target.astype(np.float64))
    order = np.argsort(b2, kind="stable")
    # position j + 1024*k holds the target of sorted rank 16*j + k
    perm = np.empty(N_TGT, np.int64)
    jj, kk = np.meshgrid(np.arange(FOLD_TO), np.arange(FOLD_S), indexing="ij")
    perm[jj + FOLD_TO * kk] = order[FOLD_S * jj + kk]

    a_t = np.ascontiguousarray(2.0 * x_pred.T).astype(ml_dtypes.bfloat16)
    b_t = np.ascontiguousarray(x_target[perm].T).astype(ml_dtypes.bfloat16)
    nb2c_row = (-b2[order[::FOLD_S]]).astype(np.float16)  # -min b2 per chunk
    nb2c = np.ascontiguousarray(np.broadcast_to(nb2c_row[None, :], (128, FOLD_TO)))
    cand_map = order.reshape(FOLD_TO, FOLD_S)  # chunk j -> target ids
    return a_t, b_t, nb2c, cand_map


def _host_finish(x_pred, x_target, chunk_val, cand_map, k):
    """chunk_val: [N_PRED, FOLD_TO] float32, C(j) >= best m in chunk j."""
    n = x_pred.shape[0]
    ch = np.argpartition(-chunk_val, TOP_CHUNKS, axis=1)[:, :TOP_CHUNKS]
    tid = cand_map[ch].reshape(n, TOP_CHUNKS * FOLD_S)

    a64 = x_pred.astype(np.float64)
    b64 = x_target.astype(np.float64)
    a2 = np.einsum("ij,ij->i", a64, a64)
    b2 = np.einsum("ij,ij->i", b64, b64)

    vals = np.empty((n, k))
    B = 1024
    for s in range(0, n, B):
        t = tid[s : s + B]
        bg = b64[t]  # [B, C, DIM]
        dots = np.einsum("rd,rcd->rc", a64[s : s + B], bg, optimize=True)
        d2 = a2[s : s + B, None] + b2[t] - 2.0 * dots
        vals[s : s + B] = np.partition(d2, k - 1, axis=1)[:, :k]
    d = np.sqrt(np.maximum(vals, 0.0))
    return np.float32(np.maximum(d - HINGE, 0.0).mean(dtype=np.float64))


def _host_exact(x_pred, x_target, k):
    """Exact fallback (never expected in practice)."""
    a = x_pred.astype(np.float32)
    b = x_target.astype(np.float32)
    a2 = np.sum(a * a, axis=1)[:, None]
    b2 = np.sum(b * b, axis=1)[None, :]
    out = np.empty((a.shape[0], k), np.float64)
    B = 1024
    for s in range(0, a.shape[0], B):
        d2 = a2[s : s + B] + b2 - 2.0 * (a[s : s + B] @ b.T)
        out[s : s + B] = np.partition(d2, k - 1, axis=1)[:, :k].astype(np.float64)
    d = np.sqrt(np.maximum(out, 0.0))
    return np.float32(np.maximum(d - HINGE, 0.0).mean(dtype=np.float64))


def kernel(x_pred, x_target, top_k=5, _want_results=False):
    from concourse.bass_utils import run_bass_kernel_spmd

    x_pred = np.asarray(x_pred, dtype=np.float32)
    x_target = np.asarray(x_target, dtype=np.float32)
    k = int(top_k)
    if (
        k > TOP_CHUNKS
        or x_pred.shape != (N_PRED, DIM)
        or x_target.shape != (N_TGT, DIM)
    ):
        return _host_exact(x_pred, x_target, k)

    nc = _get_nc()
    a_t_full, b_t, nb2c, cand_map = _prep(x_pred, x_target)

    in_maps = []
    for c in range(N_CORES):
        in_maps.append(
            {
                "a_t": np.ascontiguousarray(
                    a_t_full[:, c * ROWS_PER_CORE : (c + 1) * ROWS_PER_CORE]
                ),
                "b_t": b_t,
                "nb2c": nb2c,
            }
        )

    res = run_bass_kernel_spmd(nc, in_maps, list(range(N_CORES)))
    chunk_val = np.concatenate(
        [
            res.results[c]["cmx"].reshape(ROWS_PER_CORE, FOLD_TO)
            for c in range(N_CORES)
        ],
        axis=0,
    ).astype(np.float32)
    out = _host_finish(x_pred, x_target, chunk_val, cand_map, k)
    if _want_results:
        return out, res
    return out


# revision 12
# speedup vs baseline: 1.4495x; 1.0233x over previous
"""Trainium2 Bass kernel for nn_DensityLoss (retrieval kNN hinge loss).

Computes mean(relu(topk_smallest_dist(x_pred, x_target, k) - 1.0)).

Strategy (8 NeuronCores, SPMD, x_pred rows sharded):
  - Host sorts targets by ||b||^2 and lays them out so each of 1024
    "fold chunks" (strided positions {j + 1024k}) holds 16 targets of
    nearly equal ||b||^2. Host pre-transposes to [dim, n] (factor 2 of the
    cross term folded into a).
  - Device per core: TensorE computes 2*a.b (bf16 ops, fp32 PSUM);
    ScalarE casts PSUM into an fp16 SBUF slab; DVE runs an elementwise-max
    fold tree 16384 -> 2048 (fp16 2x mode; two fold pairs read their second
    operand straight from PSUM, skipping two ScalarE copies per rowtile).
  - The [rows, 2048] half-chunk maxima DMA back to host. Host finishes the
    last fold level, adds the per-chunk -min||b||^2, picks the top-12
    chunks per row (a chunk holding a true top-5 target ranks <= 5 +
    b2-spread noise; 12 leaves a wide margin), rescores the 12*16 = 192
    candidates exactly in float64, takes top-k, hinges, averages.
"""

import numpy as np

N_CORES = 8
N_PRED = 8192
N_TGT = 16384
DIM = 128
ROWS_PER_CORE = N_PRED // N_CORES  # 1024
ROWTILES = ROWS_PER_CORE // 128    # 8
BANK = 512                         # fp32 PSUM bank, matmul max N
GROUP = 4                          # banks per PSUM tile
N_GROUPS = N_TGT // (BANK * GROUP)  # 8 groups of 2048 targets per rowtile
OUT_W = 2048                       # fold-tree output width (device side)
FOLD_TO = 1024                     # chunk count (final, after host fold)
FOLD_S = N_TGT // FOLD_TO          # 16 targets per fold chunk
TOP_CHUNKS = 12
HINGE = 1.0

_CACHE = {}


def _build_nc():
    import concourse.bacc as bacc
    import concourse.bass as bass
    import concourse.mybir as mybir
    import concourse.tile as tile

    dt = mybir.dt
    nc = bacc.Bacc(
        "TRN2",
        target_bir_lowering=False,
        debug=False,
        num_devices=N_CORES,
    )
    a_t = nc.dram_tensor("a_t", [DIM, ROWS_PER_CORE], dt.bfloat16, kind="ExternalInput")
    b_t = nc.dram_tensor("b_t", [DIM, N_TGT], dt.bfloat16, kind="ExternalInput")
    cmx = nc.dram_tensor(
        "cmx", [ROWTILES, 128, OUT_W], dt.float16, kind="ExternalOutput"
    )

    Gg = BANK * GROUP  # 2048 targets per PSUM tile
    # slab holds fp16 2a.b for groups 0,1,2,3,6,7; fold pairs (0,4) and
    # (1,5) read groups 4/5 straight from their PSUM tiles (PSUM+SBUF is
    # the only operand mix the BIR verifier accepts).
    soff = {0: 0, 1: 2048, 2: 4096, 3: 6144, 6: 8192, 7: 10240}

    with tile.TileContext(nc) as tc:
        with (
            tc.tile_pool(name="const", bufs=1) as cpool,
            tc.tile_pool(name="psum", bufs=2, space="PSUM") as ppool,
            tc.tile_pool(name="slab", bufs=3) as spool,
            tc.tile_pool(name="fold", bufs=2) as fpool,
        ):
            bt_sb = cpool.tile([DIM, N_TGT], dt.bfloat16)
            at_sb = cpool.tile([DIM, ROWS_PER_CORE], dt.bfloat16)

            nc.sync.dma_start(out=at_sb[:], in_=a_t[:])
            # Fine-grained slices so the first matmuls start early.
            for s in range(N_TGT // BANK):
                sl = bass.ts(s, BANK)
                nc.sync.dma_start(out=bt_sb[:, sl], in_=b_t[:, sl])

            for rt in range(ROWTILES):
                lhsT = at_sb[:, bass.ts(rt, 128)]
                slab = spool.tile([128, 6 * Gg], dt.float16)
                f1 = fpool.tile([128, N_TGT // 2], dt.float16, tag="f1")
                tiles = {}

                def mains(g, tiles=tiles, lhsT=lhsT):
                    ps = ppool.tile([128, Gg], dt.float32)
                    tiles[g] = ps
                    for j in range(GROUP):
                        c = g * GROUP + j
                        nc.tensor.matmul(
                            ps[:, bass.ts(j, BANK)],
                            lhsT,
                            bt_sb[:, bass.ts(c, BANK)],
                            start=True,
                            stop=True,
                        )

                def evac(g, slab=slab, tiles=tiles):
                    nc.scalar.copy(
                        slab[:, soff[g] : soff[g] + Gg], tiles.pop(g)[:]
                    )

                for g in (0, 1, 2, 3):
                    mains(g)
                    evac(g)
                for g in (4, 5):
                    # fold pairs (0,4) / (1,5): second operand from PSUM
                    mains(g)
                    p = g - 4
                    nc.vector.tensor_max(
                        f1[:, bass.ts(p, Gg)],
                        slab[:, soff[p] : soff[p] + Gg],
                        tiles.pop(g)[:],
                    )
                for g in (6, 7):
                    mains(g)
                    evac(g)
                    p = g - 4
                    nc.vector.tensor_max(
                        f1[:, bass.ts(p, Gg)],
                        slab[:, soff[p] : soff[p] + Gg],
                        slab[:, soff[g] : soff[g] + Gg],
                    )
                f2 = fpool.tile([128, 4096], dt.float16, tag="f2")
                nc.vector.tensor_max(f2[:], f1[:, 0:4096], f1[:, 4096:8192])
                f3 = fpool.tile([128, OUT_W], dt.float16, tag="f3")
                nc.vector.tensor_max(f3[:], f2[:, 0:OUT_W], f2[:, OUT_W : 2 * OUT_W])
                nc.sync.dma_start(out=cmx[rt], in_=f3[:])

    nc.compile()
    return nc


def _get_nc():
    if "nc" not in _CACHE:
        _CACHE["nc"] = _build_nc()
    return _CACHE["nc"]


def _prep(x_pred, x_target):
    """Host-side layout: sort targets by b2, stride into fold chunks."""
    import ml_dtypes

    b2 = np.einsum("ij,ij->i", x_target.astype(np.float64), x_target.astype(np.float64))
    order = np.argsort(b2, kind="stable")
    # position j + 1024*k holds the target of sorted rank 16*j + k
    perm = np.empty(N_TGT, np.int64)
    jj, kk = np.meshgrid(np.arange(FOLD_TO), np.arange(FOLD_S), indexing="ij")
    perm[jj + FOLD_TO * kk] = order[FOLD_S * jj + kk]

    a_t = np.ascontiguousarray(2.0 * x_pred.T).astype(ml_dtypes.bfloat16)
    b_t = np.ascontiguousarray(x_target[perm].T).astype(ml_dtypes.bfloat16)
    nb2c_row = (-b2[order[::FOLD_S]]).astype(np.float32)  # -min b2 per chunk
    cand_map = order.reshape(FOLD_TO, FOLD_S)  # chunk j -> target ids
    return a_t, b_t, nb2c_row, cand_map


def _host_finish(x_pred, x_target, half_max, nb2c_row, cand_map, k):
    """half_max: [N_PRED, 2048] fp32; halves j and j+1024 belong to chunk j.
    C(j) = max(halves) - min b2 >= best (2 a.b - b2) in chunk j."""
    n = x_pred.shape[0]
    chunk_val = np.maximum(half_max[:, :FOLD_TO], half_max[:, FOLD_TO:]) + nb2c_row
    ch = np.argpartition(-chunk_val, TOP_CHUNKS, axis=1)[:, :TOP_CHUNKS]
    tid = cand_map[ch].reshape(n, TOP_CHUNKS * FOLD_S)

    a64 = x_pred.astype(np.float64)
    b64 = x_target.astype(np.float64)
    a2 = np.einsum("ij,ij->i", a64, a64)
    b2 = np.einsum("ij,ij->i", b64, b64)

    vals = np.empty((n, k))
    B = 1024
    for s in range(0, n, B):
        t = tid[s : s + B]
        bg = b64[t]  # [B, C, DIM]
        dots = np.einsum("rd,rcd->rc", a64[s : s + B], bg, optimize=True)
        d2 = a2[s : s + B, None] + b2[t] - 2.0 * dots
        vals[s : s + B] = np.partition(d2, k - 1, axis=1)[:, :k]
    d = np.sqrt(np.maximum(vals, 0.0))
    return np.float32(np.maximum(d - HINGE, 0.0).mean(dtype=np.float64))


def _host_exact(x_pred, x_target, k):
    """Exact fallback (never expected in practice)."""
    a = x_pred.astype(np.float32)
    b = x_target.astype(np.float32)
    a2 = np.sum(a * a, axis=1)[:, None]
    b2 = np.sum(b * b, axis=1)[None, :]
    out = np.empty((a.shape[0], k), np.float64)
    B = 1024
    for s in range(0, a.shape[0], B):
        d2 = a2[s : s + B] + b2 - 2.0 * (a[s : s + B] @ b.T)
        out[s : s + B] = np.partition(d2, k - 1, axis=1)[:, :k].astype(np.float64)
    d = np.sqrt(np.maximum(out, 0.0))
    return np.float32(np.maximum(d - HINGE, 0.0).mean(dtype=np.float64))


def kernel(x_pred, x_target, top_k=5, _want_results=False):
    from concourse.bass_utils import run_bass_kernel_spmd

    x_pred = np.asarray(x_pred, dtype=np.float32)
    x_target = np.asarray(x_target, dtype=np.float32)
    k = int(top_k)
    if (
        k > TOP_CHUNKS
        or x_pred.shape != (N_PRED, DIM)
        or x_target.shape != (N_TGT, DIM)
    ):
        return _host_exact(x_pred, x_target, k)

    nc = _get_nc()
    a_t_full, b_t, nb2c_row, cand_map = _prep(x_pred, x_target)

    in_maps = []
    for c in range(N_CORES):
        in_maps.append(
            {
                "a_t": np.ascontiguousarray(
                    a_t_full[:, c * ROWS_PER_CORE : (c + 1) * ROWS_PER_CORE]
                ),
                "b_t": b_t,
            }
        )

    res = run_bass_kernel_spmd(nc, in_maps, list(range(N_CORES)))
    half_max = np.concatenate(
        [
            res.results[c]["cmx"].reshape(ROWS_PER_CORE, OUT_W)
            for c in range(N_CORES)
        ],
        axis=0,
    ).astype(np.float32)
    out = _host_finish(x_pred, x_target, half_max, nb2c_row, cand_map, k)
    if _want_results:
        return out, res
    return out


# revision 13
# speedup vs baseline: 1.4607x; 1.0077x over previous
"""Trainium2 Bass kernel for nn_DensityLoss (retrieval kNN hinge loss).

Computes mean(relu(topk_smallest_dist(x_pred, x_target, k) - 1.0)).

Strategy (8 NeuronCores, SPMD, x_pred rows sharded):
  - Host sorts targets by ||b||^2 and lays them out so each of 1024
    "fold chunks" (strided positions {j + 1024k}) holds 16 targets of
    nearly equal ||b||^2. Host pre-transposes to [dim, n] (factor 2 of the
    cross term folded into a).
  - Device per core: TensorE computes 2*a.b (bf16 ops, fp32 PSUM);
    ScalarE casts PSUM into an fp16 SBUF slab; DVE runs an elementwise-max
    fold tree 16384 -> 2048 (fp16 2x mode; two fold pairs read their second
    operand straight from PSUM, skipping two ScalarE copies per rowtile).
  - The [rows, 2048] half-chunk maxima DMA back to host. Host finishes the
    last fold level, adds the per-chunk -min||b||^2, picks the top-12
    chunks per row (a chunk holding a true top-5 target ranks <= 5 +
    b2-spread noise; 12 leaves a wide margin), rescores the 12*16 = 192
    candidates exactly in float64, takes top-k, hinges, averages.
"""

import numpy as np

N_CORES = 8
N_PRED = 8192
N_TGT = 16384
DIM = 128
ROWS_PER_CORE = N_PRED // N_CORES  # 1024
ROWTILES = ROWS_PER_CORE // 128    # 8
BANK = 512                         # fp32 PSUM bank, matmul max N
GROUP = 4                          # banks per PSUM tile
N_GROUPS = N_TGT // (BANK * GROUP)  # 8 groups of 2048 targets per rowtile
OUT_W = 2048                       # fold-tree output width (device side)
FOLD_TO = 1024                     # chunk count (final, after host fold)
FOLD_S = N_TGT // FOLD_TO          # 16 targets per fold chunk
TOP_CHUNKS = 12
HINGE = 1.0

_CACHE = {}


def _build_nc():
    import concourse.bacc as bacc
    import concourse.bass as bass
    import concourse.mybir as mybir
    import concourse.tile as tile

    dt = mybir.dt
    nc = bacc.Bacc(
        "TRN2",
        target_bir_lowering=False,
        debug=False,
        num_devices=N_CORES,
    )
    a_t = nc.dram_tensor("a_t", [DIM, ROWS_PER_CORE], dt.bfloat16, kind="ExternalInput")
    b_t = nc.dram_tensor("b_t", [DIM, N_TGT], dt.bfloat16, kind="ExternalInput")
    cmx = nc.dram_tensor(
        "cmx", [ROWTILES, 128, OUT_W], dt.float16, kind="ExternalOutput"
    )

    Gg = BANK * GROUP  # 2048 targets per PSUM tile
    # slab holds fp16 2a.b for groups 0,1,2,3,6,7; fold pairs (0,4) and
    # (1,5) read groups 4/5 straight from their PSUM tiles (PSUM+SBUF is
    # the only operand mix the BIR verifier accepts).
    soff = {0: 0, 1: 2048, 2: 4096, 3: 6144, 6: 8192, 7: 10240}

    with tile.TileContext(nc) as tc:
        with (
            tc.tile_pool(name="const", bufs=1) as cpool,
            tc.tile_pool(name="psum", bufs=2, space="PSUM") as ppool,
            tc.tile_pool(name="slab", bufs=3) as spool,
            tc.tile_pool(name="fold", bufs=2) as fpool,
        ):
            bt_sb = cpool.tile([DIM, N_TGT], dt.bfloat16)
            at_sb = cpool.tile([DIM, ROWS_PER_CORE], dt.bfloat16)

            nc.sync.dma_start(out=at_sb[:], in_=a_t[:])
            # Fine-grained slices so the first matmuls start early.
            for s in range(N_TGT // BANK):
                sl = bass.ts(s, BANK)
                nc.sync.dma_start(out=bt_sb[:, sl], in_=b_t[:, sl])

            pending_tail = None

            def run_tail():
                nonlocal pending_tail
                if pending_tail is not None:
                    pending_tail()
                    pending_tail = None

            for rt in range(ROWTILES):
                lhsT = at_sb[:, bass.ts(rt, 128)]
                slab = spool.tile([128, 6 * Gg], dt.float16)
                f1 = fpool.tile([128, N_TGT // 2], dt.float16, tag="f1")
                tiles = {}

                def mains(g, tiles=tiles, lhsT=lhsT):
                    ps = ppool.tile([128, Gg], dt.float32)
                    tiles[g] = ps
                    for j in range(GROUP):
                        c = g * GROUP + j
                        nc.tensor.matmul(
                            ps[:, bass.ts(j, BANK)],
                            lhsT,
                            bt_sb[:, bass.ts(c, BANK)],
                            start=True,
                            stop=True,
                        )

                def evac(g, slab=slab, tiles=tiles):
                    nc.scalar.copy(
                        slab[:, soff[g] : soff[g] + Gg], tiles.pop(g)[:]
                    )

                for g in (0, 1, 2, 3):
                    mains(g)
                    evac(g)
                    if g == 1:
                        # previous rowtile's fold tail overlaps this
                        # rowtile's matmul/copy stream
                        run_tail()
                for g in (4, 5):
                    # fold pairs (0,4) / (1,5): second operand from PSUM
                    mains(g)
                    p = g - 4
                    nc.vector.tensor_max(
                        f1[:, bass.ts(p, Gg)],
                        slab[:, soff[p] : soff[p] + Gg],
                        tiles.pop(g)[:],
                    )
                for g in (6, 7):
                    mains(g)
                    evac(g)
                    p = g - 4
                    nc.vector.tensor_max(
                        f1[:, bass.ts(p, Gg)],
                        slab[:, soff[p] : soff[p] + Gg],
                        slab[:, soff[g] : soff[g] + Gg],
                    )

                def tail(rt=rt, f1=f1):
                    f2 = fpool.tile([128, 4096], dt.float16, tag="f2")
                    nc.vector.tensor_max(f2[:], f1[:, 0:4096], f1[:, 4096:8192])
                    f3 = fpool.tile([128, OUT_W], dt.float16, tag="f3")
                    nc.vector.tensor_max(
                        f3[:], f2[:, 0:OUT_W], f2[:, OUT_W : 2 * OUT_W]
                    )
                    half = OUT_W // 2
                    nc.sync.dma_start(out=cmx[rt][:, 0:half], in_=f3[:, 0:half])
                    nc.sync.dma_start(out=cmx[rt][:, half:], in_=f3[:, half:])

                pending_tail = tail
            run_tail()

    nc.compile()
    return nc


def _get_nc():
    if "nc" not in _CACHE:
        _CACHE["nc"] = _build_nc()
    return _CACHE["nc"]


def _prep(x_pred, x_target):
    """Host-side layout: sort targets by b2, stride into fold chunks."""
    import ml_dtypes

    b2 = np.einsum("ij,ij->i", x_target.astype(np.float64), x_target.astype(np.float64))
    order = np.argsort(b2, kind="stable")
    # position j + 1024*k holds the target of sorted rank 16*j + k
    perm = np.empty(N_TGT, np.int64)
    jj, kk = np.meshgrid(np.arange(FOLD_TO), np.arange(FOLD_S), indexing="ij")
    perm[jj + FOLD_TO * kk] = order[FOLD_S * jj + kk]

    a_t = np.ascontiguousarray(2.0 * x_pred.T).astype(ml_dtypes.bfloat16)
    b_t = np.ascontiguousarray(x_target[perm].T).astype(ml_dtypes.bfloat16)
    nb2c_row = (-b2[order[::FOLD_S]]).astype(np.float32)  # -min b2 per chunk
    cand_map = order.reshape(FOLD_TO, FOLD_S)  # chunk j -> target ids
    return a_t, b_t, nb2c_row, cand_map


def _host_finish(x_pred, x_target, half_max, nb2c_row, cand_map, k):
    """half_max: [N_PRED, 2048] fp32; halves j and j+1024 belong to chunk j.
    C(j) = max(halves) - min b2 >= best (2 a.b - b2) in chunk j."""
    n = x_pred.shape[0]
    chunk_val = np.maximum(half_max[:, :FOLD_TO], half_max[:, FOLD_TO:]) + nb2c_row
    ch = np.argpartition(-chunk_val, TOP_CHUNKS, axis=1)[:, :TOP_CHUNKS]
    tid = cand_map[ch].reshape(n, TOP_CHUNKS * FOLD_S)

    a64 = x_pred.astype(np.float64)
    b64 = x_target.astype(np.float64)
    a2 = np.einsum("ij,ij->i", a64, a64)
    b2 = np.einsum("ij,ij->i", b64, b64)

    vals = np.empty((n, k))
    B = 1024
    for s in range(0, n, B):
        t = tid[s : s + B]
        bg = b64[t]  # [B, C, DIM]
        dots = np.einsum("rd,rcd->rc", a64[s : s + B], bg, optimize=True)
        d2 = a2[s : s + B, None] + b2[t] - 2.0 * dots
        vals[s : s + B] = np.partition(d2, k - 1, axis=1)[:, :k]
    d = np.sqrt(np.maximum(vals, 0.0))
    return np.float32(np.maximum(d - HINGE, 0.0).mean(dtype=np.float64))


def _host_exact(x_pred, x_target, k):
    """Exact fallback (never expected in practice)."""
    a = x_pred.astype(np.float32)
    b = x_target.astype(np.float32)
    a2 = np.sum(a * a, axis=1)[:, None]
    b2 = np.sum(b * b, axis=1)[None, :]
    out = np.empty((a.shape[0], k), np.float64)
    B = 1024
    for s in range(0, a.shape[0], B):
        d2 = a2[s : s + B] + b2 - 2.0 * (a[s : s + B] @ b.T)
        out[s : s + B] = np.partition(d2, k - 1, axis=1)[:, :k].astype(np.float64)
    d = np.sqrt(np.maximum(out, 0.0))
    return np.float32(np.maximum(d - HINGE, 0.0).mean(dtype=np.float64))


def kernel(x_pred, x_target, top_k=5, _want_results=False):
    from concourse.bass_utils import run_bass_kernel_spmd

    x_pred = np.asarray(x_pred, dtype=np.float32)
    x_target = np.asarray(x_target, dtype=np.float32)
    k = int(top_k)
    if (
        k > TOP_CHUNKS
        or x_pred.shape != (N_PRED, DIM)
        or x_target.shape != (N_TGT, DIM)
    ):
        return _host_exact(x_pred, x_target, k)

    nc = _get_nc()
    a_t_full, b_t, nb2c_row, cand_map = _prep(x_pred, x_target)

    in_maps = []
    for c in range(N_CORES):
        in_maps.append(
            {
                "a_t": np.ascontiguousarray(
                    a_t_full[:, c * ROWS_PER_CORE : (c + 1) * ROWS_PER_CORE]
                ),
                "b_t": b_t,
            }
        )

    res = run_bass_kernel_spmd(nc, in_maps, list(range(N_CORES)))
    half_max = np.concatenate(
        [
            res.results[c]["cmx"].reshape(ROWS_PER_CORE, OUT_W)
            for c in range(N_CORES)
        ],
        axis=0,
    ).astype(np.float32)
    out = _host_finish(x_pred, x_target, half_max, nb2c_row, cand_map, k)
    if _want_results:
        return out, res
    return out
